# revision 81
# baseline (speedup 1.0000x reference)
"""GNN NodeModel kernel for 8 Trainium2 NeuronCores (Bass/Tile) — fp8 V5.

Design (per core, nodes sharded 2500/core, edges sorted by destination).
All big matmuls run in fp8-e4m3 DoubleRow perf mode (256-deep contraction,
0.5 cycles/row — 4x the bf16 instruction cost per unit work); f32 PSUM.

  Phase 1 (xw): dedup sources — xw = x8[u] @ W1a_top8 + b1a8 for the core's
    ~8000 unique source nodes (host supplies x8[u] chunk-major transposed
    fp8), bias added via a K=1 DoubleRow ones-matmul; fp8 rows to a DRAM
    scratch. pw rotates over 3 PSUM buffers (psBig x2 + psPr, idle until
    phase 2); the PSUM->SBUF casts split per half across DVE+ACT so the
    copy chain hides inside the next ug's PE window.
  Phase 2 (fused edge + segment): nodes packed into 20 tiles of <=128
    nodes; each tile owns G_q 128-edge groups, processed in PAIRS so the
    segment matmuls can also run DoubleRow (256-edge contraction).
    Per group: gather xw rows (indirect DMA, fp8); ph = ea8 @ W1a_bot8
    (4 DR matmuls x 2 halves); gt = ph + xw_g (DVE); g = relu(gt) -> fp8
    half of the pair tile. Per pair: prT[:,k,:] += [gA|gB].T @ [spA|spB]
    (DR, deferred 3 pairs so the gather/add/relu chain stays off the PE
    critical path), where sp holds fp8(1/deg) — the accumulated result is
    the mean, feature-major, ready for MLP2.
  Phase 3 (mlp2, interleaved per 2 node tiles): folded node MLP2:
      o1 = relu(x8@B1_8s16 + xr16@B1_8 + x8@B1_r16 + rm8@(16W3)_8
                + (mask x 16u, as a 5th DR chunk-pair) + 16 b2a)
      out = o1 @ W2b + b2b     [x@B1 and o1@W2b both exact via 3-term
                                fp8 residuals: W2b = w2b8 + w2br64/64 and
                                o1 = (o18s4 + rr4)/64, PSUM at 64x]
    W3 = W1b@W2a_bot, u = b1b@W2a_bot (host-folded). Output written
    chunk-major transposed bf16, unpacked on host.

  Queues: SP = input streams (xu/eat/sp/xt superblocks, xw writes);
  Pool = weight loads (gated mid-phase-1 off xw_dram rows to dodge the
  serial DMA_ENGINES device) + indirect gathers + out writes; ACT/DVE =
  casts, adds, relus. TimelineSim-modeled 332.4us/core (baseline 810.6).
"""

import sys

sys.path.insert(0, "/opt/trn_rl_repo")

from contextlib import ExitStack

import numpy as np
import ml_dtypes

import concourse.bass as bass
import concourse.tile as tile
from concourse import bacc, mybir
from concourse.bass_utils import run_bass_kernel_spmd

N = 20000
E = 80000
D = 1024
C = 8
NPC = N // C      # 2500 nodes per core
NT = 20           # node tiles per core
NP = NT * 128     # 2560 padded node slots
KC = D // 128     # 8 feature chunks of 128
K4 = D // 256     # 4 feature chunks of 256 (DoubleRow)
F32 = mybir.dt.float32
BF16 = mybir.dt.bfloat16
FP8 = mybir.dt.float8e4
I32 = mybir.dt.int32
BF = ml_dtypes.bfloat16
E4M3 = ml_dtypes.float8_e4m3
DR = mybir.MatmulPerfMode.DoubleRow

AF = mybir.ActivationFunctionType

_PROGRAM_CACHE = {}
DEFAULT_PROGRAM_ARGS = (80, 63, (4,) * 20)


def _pair_schedule(pattern):
    """Groups per node tile paired up: list of (q, [jjA] or [jjA, jjB])."""
    pairs = []
    jj = 0
    for q in range(NT):
        js = list(range(jj, jj + pattern[q]))
        jj += pattern[q]
        for i in range(0, len(js), 2):
            pairs.append((q, js[i : i + 2]))
    return pairs


def _build_program(NG, GU, pattern):
    """NG: total 128-edge groups/core (sum of pattern); GU: 128-row groups of
    unique source nodes; pattern: groups per node tile (len NT)."""
    assert sum(pattern) == NG and len(pattern) == NT
    UP = GU * 128
    S = NG * 128
    pairs = _pair_schedule(pattern)
    NPAIR = len(pairs)

    nc = bacc.Bacc("TRN2", target_bir_lowering=False, debug=False, num_devices=C)

    # ---- DRAM I/O (fp8 tensors are chunk-major: [p, c, n] = M[128c+p, n]) --
    xtu_d = nc.dram_tensor("xtu_d", [128, KC, UP], FP8, kind="ExternalInput").ap()
    eat_d = nc.dram_tensor("eat_d", [128, KC, S], FP8, kind="ExternalInput").ap()
    sp_d = nc.dram_tensor("sp_d", [128, NPAIR, 2, 128], FP8, kind="ExternalInput").ap()
    srcidx = nc.dram_tensor("srcidx", [128, NG], I32, kind="ExternalInput").ap()
    x8t_d = nc.dram_tensor("x8t_d", [128, NT // 2, KC, 256], FP8, kind="ExternalInput").ap()
    xr16t_d = nc.dram_tensor("xr16t_d", [128, NT // 2, KC, 256], FP8, kind="ExternalInput").ap()
    maskv8 = nc.dram_tensor("maskv8", [1, NP], FP8, kind="ExternalInput").ap()
    ones2_d = nc.dram_tensor("ones2_d", [1, 2, 128], FP8, kind="ExternalInput").ap()
    w1a_d = nc.dram_tensor("w1a_d", [128, 2 * KC, D], FP8, kind="ExternalInput").ap()
    b1s_d = nc.dram_tensor("b1s_d", [128, KC, D], FP8, kind="ExternalInput").ap()
    b1p_d = nc.dram_tensor("b1p_d", [128, KC, D], FP8, kind="ExternalInput").ap()
    b1r_d = nc.dram_tensor("b1r_d", [128, KC, D], FP8, kind="ExternalInput").ap()
    w3s_d = nc.dram_tensor("w3s_d", [128, KC + 2, D], FP8, kind="ExternalInput").ap()
    w2b8_d = nc.dram_tensor("w2b8_d", [128, KC, D], FP8, kind="ExternalInput").ap()
    w2br_d = nc.dram_tensor("w2br_d", [128, KC, D], FP8, kind="ExternalInput").ap()
    b1a2_d = nc.dram_tensor("b1a2_d", [1, 2, D], FP8, kind="ExternalInput").ap()
    b2a_d = nc.dram_tensor("b2a_d", [128, KC], F32, kind="ExternalInput").ap()
    b2b_d = nc.dram_tensor("b2b_d", [128, KC], F32, kind="ExternalInput").ap()
    out_myT = nc.dram_tensor("out_myT", [128, KC, NP], BF16, kind="ExternalOutput").ap()
    xw_dram = nc.dram_tensor("xw_scratch", [UP, D], FP8).ap()

    NSB_U = (GU + 3) // 4   # xTu superblocks of 512 cols
    NSB_P = (NPAIR + 3) // 4  # sp superblocks of 4 pairs

    with tile.TileContext(nc) as tc, ExitStack() as ctx:
        cpool = ctx.enter_context(tc.tile_pool(name="consts", bufs=1))
        wpool = ctx.enter_context(tc.tile_pool(name="weights", bufs=1))
        sxu = ctx.enter_context(tc.tile_pool(name="sxu", bufs=4))
        sea = ctx.enter_context(tc.tile_pool(name="sea", bufs=3))
        ssp = ctx.enter_context(tc.tile_pool(name="ssp", bufs=2))
        pgat = ctx.enter_context(tc.tile_pool(name="pgat", bufs=12))
        pg = ctx.enter_context(tc.tile_pool(name="pg", bufs=4))
        pg2 = ctx.enter_context(tc.tile_pool(name="pg2", bufs=6))
        pxw = ctx.enter_context(tc.tile_pool(name="pxw", bufs=24))
        prm = ctx.enter_context(tc.tile_pool(name="prm", bufs=2))
        pxt = ctx.enter_context(tc.tile_pool(name="pxt", bufs=2))
        po1 = ctx.enter_context(tc.tile_pool(name="po1", bufs=2))
        psBig = ctx.enter_context(tc.tile_pool(name="psBig", bufs=2, space="PSUM"))
        psPr = ctx.enter_context(tc.tile_pool(name="psPr", bufs=1, space="PSUM"))
        psMm2 = ctx.enter_context(tc.tile_pool(name="psMm2", bufs=2, space="PSUM"))

        # ---- phase-1-critical loads first, in consumption order: the
        # k4-chunk pairs arrive as the PE's contraction loop needs them;
        # ones2/b1a2 only matter at the 5th matmul (bias)
        w1a_sb = wpool.tile([128, 2 * KC, D], FP8, tag="w1a")
        hi0 = min(512, UP)
        xu = sxu.tile([128, KC, 512], FP8, tag="xu", name="xu0")
        # spread the startup-critical loads across queues so the issue
        # overhead parallelizes (transfers still serialize on DMA_ENGINES)
        ones2_sb = cpool.tile([1, 2, 128], FP8, tag="ones2")
        nc.scalar.dma_start(ones2_sb[:], ones2_d[:])
        b1a2_sb = cpool.tile([1, 2, D], FP8, tag="b1a2")
        nc.scalar.dma_start(b1a2_sb[:], b1a2_d[:])
        nc.sync.dma_start(w1a_sb[:, 0:2, :], w1a_d[:, 0:2, :])
        nc.sync.dma_start(xu[:, 0:2, :hi0], xtu_d[:, 0:2, :hi0])
        nc.scalar.dma_start(w1a_sb[:, 2:4, :], w1a_d[:, 2:4, :])
        nc.gpsimd.dma_start(w1a_sb[:, 4:6, :], w1a_d[:, 4:6, :])
        nc.sync.dma_start(w1a_sb[:, 6:KC, :], w1a_d[:, 6:KC, :])
        nc.gpsimd.dma_start(xu[:, 2:4, :hi0], xtu_d[:, 2:4, :hi0])
        nc.sync.dma_start(xu[:, 4:KC, :hi0], xtu_d[:, 4:KC, :hi0])
        xu_tiles = {0: xu}

        def load_xu(sb):
            if sb >= NSB_U or sb in xu_tiles:
                return
            hi = min(512, UP - 512 * sb)
            t = sxu.tile([128, KC, 512], FP8, tag="xu", name=f"xu{sb}")
            nc.sync.dma_start(t[:, :, :hi], xtu_d[:, :, 512 * sb : 512 * sb + hi])
            xu_tiles[sb] = t

        load_xu(1)
        load_xu(2)
        NSB_E = (NG + 3) // 4
        eat_tiles = {}

        def load_eat(sb):
            if sb >= NSB_E or sb in eat_tiles:
                return
            hi = min(512, S - 512 * sb)
            t = sea.tile([128, KC, 512], FP8, tag="ea", name=f"ea{sb}")
            nc.sync.dma_start(t[:, :, :hi], eat_d[:, :, 512 * sb : 512 * sb + hi])
            eat_tiles[sb] = t
        # srcidx early: first gather fires right at phase-2 start
        srcidx_sb = cpool.tile([128, NG], I32, tag="srcidx")
        nc.sync.dma_start(srcidx_sb[:], srcidx[:])

        # ---- phase 2/3 weights: Pool queue, gated behind a dummy copy that
        # depends on ug 6's output so their transfers don't contend with the
        # startup-critical xu/w1a loads on the (exclusive) DMA_ENGINES device
        b2a_sb = cpool.tile([128, KC], F32, tag="b2a")
        b2b_sb = cpool.tile([128, KC], F32, tag="b2b")
        b1s_sb = wpool.tile([128, KC, D], FP8, tag="b1s")
        b1p_sb = wpool.tile([128, KC, D], FP8, tag="b1p")
        b1r_sb = wpool.tile([128, KC, D], FP8, tag="b1r")
        w3s_sb = wpool.tile([128, KC + 2, D], FP8, tag="w3s")
        w2b8_sb = wpool.tile([128, KC, D], FP8, tag="w2b8")
        w2br_sb = wpool.tile([128, KC, D], FP8, tag="w2br")
        zeros_sb = cpool.tile([128, 256], BF16, tag="zeros")
        nc.vector.memset(zeros_sb[:], 0.0)

        wgate = cpool.tile([1, 64], FP8, tag="wgate")

        wgate2 = cpool.tile([1, 64], FP8, tag="wgate2")

        def emit_weight_loads(gate_ug):
            # phase-2-start-critical only (w1a bottom + small consts).
            # gate: Pool DMA reading the xw row written at gate_ug — releases
            # as soon as that SP write lands
            nc.gpsimd.dma_start(wgate[:], xw_dram[128 * gate_ug : 128 * gate_ug + 1, 0:64])
            nc.gpsimd.dma_start(b2a_sb[:], b2a_d[:])
            nc.gpsimd.dma_start(b2b_sb[:], b2b_d[:])
            for c2 in range(0, KC, 2):
                nc.gpsimd.dma_start(w1a_sb[:, KC + c2 : KC + c2 + 2, :],
                                    w1a_d[:, KC + c2 : KC + c2 + 2, :])

        def emit_mlp2_weight_loads(gate_ug):
            # mlp2-only weights: land in phase 1's write-only final stretch
            nc.gpsimd.dma_start(wgate2[:], xw_dram[128 * gate_ug : 128 * gate_ug + 1, 0:64])
            for c2 in range(0, KC, 2):
                cs = slice(c2, c2 + 2)
                nc.gpsimd.dma_start(b1s_sb[:, cs, :], b1s_d[:, cs, :])
                nc.gpsimd.dma_start(b1p_sb[:, cs, :], b1p_d[:, cs, :])
                nc.gpsimd.dma_start(b1r_sb[:, cs, :], b1r_d[:, cs, :])
                nc.gpsimd.dma_start(w3s_sb[:, cs, :], w3s_d[:, cs, :])
                nc.gpsimd.dma_start(w2b8_sb[:, cs, :], w2b8_d[:, cs, :])
                nc.gpsimd.dma_start(w2br_sb[:, cs, :], w2br_d[:, cs, :])
            nc.gpsimd.dma_start(w3s_sb[:, KC : KC + 2, :],
                                w3s_d[:, KC : KC + 2, :])

        # ================= Phase 1: xw = x8_u @ W1a_top8 + b1a ==============
        for ug in range(GU):
            sb, col = divmod(ug, 4)
            if col == 0:
                load_xu(sb + 3)   # 3-superblock lookahead (bufs=4)
                xu = xu_tiles.pop(sb)
            # rotate over 3 PSUM buffers: psBig's two + psPr's one (psPr is
            # idle until phase 2 and its 'pr' tag has the same [128,D] f32
            # shape, so the static footprint is unchanged)
            if ug % 3 < 2:
                pw = psBig.tile([128, KC, 128], F32, tag="big", name=f"pw{ug}")
            else:
                pw = psPr.tile([128, KC, 128], F32, tag="pr", name=f"pw{ug}")
            for h in range(2):
                for k4 in range(K4):
                    nc.tensor.matmul(
                        pw[:, 4 * h : 4 * (h + 1), :],
                        xu[:, 2 * k4 : 2 * k4 + 2, 128 * col : 128 * (col + 1)],
                        w1a_sb[:, 2 * k4 : 2 * k4 + 2, 512 * h : 512 * (h + 1)],
                        start=(k4 == 0),
                        stop=False,
                        perf_mode=DR,
                    )
                nc.tensor.matmul(
                    pw[:, 4 * h : 4 * (h + 1), :],
                    ones2_sb[0:1, :, :],
                    b1a2_sb[0:1, :, 512 * h : 512 * (h + 1)],
                    start=False,
                    stop=True,
                    perf_mode=DR,
                )
            xw_sb = pxw.tile([128, D], FP8, tag="xw", name=f"xwsb{ug}")
            # quarter-split PSUM->SBUF copies on ACT+DVE, each starting as
            # soon as its PSUM half stops, so the ~0.55us sem latency plus
            # copy time hides inside the next ug's PE window
            nc.vector.tensor_copy(xw_sb[:, 0:512], pw[:, 0:4, :])
            nc.scalar.activation(xw_sb[:, 512:D], pw[:, 4:8, :], AF.Identity)
            nc.sync.dma_start(xw_dram[128 * ug : 128 * (ug + 1), :], xw_sb[:])
            if ug == min(3, GU - 1):
                emit_weight_loads(max(0, ug - 1))
            if ug == max(min(12, GU - 1), GU - 26):
                emit_mlp2_weight_loads(max(0, ug - 1))
            if ug == max(0, GU - 16):
                # preload first edge superblocks late in phase 1 so they're
                # resident the moment phase 2 starts (SP queue is in-order)
                load_eat(0)
                load_eat(1)

        # ============ Phase 2: fused edge MLP1 + paired segment means ======
        DEFER = 3           # pairs of S-matmul deferral (relu-chain cover)
        DEFER0 = 3          # deeper deferral at the phase boundary (xw-write
                            # -> gather -> add -> relu chain is ~7us)
        next_fin = 0        # next pair to finalize
        state = {}          # pi -> (g2, spv, prt, first, last, q)
        rmt_by_t2 = {}
        prt_by_q = {}
        pending_mlp2 = []
        sp_sb = None
        xt_by_t2 = {}

        def prefetch_xt(t2):
            xt8 = pxt.tile([128, KC, 256], FP8, tag="xt8", name=f"xt8_{t2}")
            nc.sync.dma_start(xt8[:], x8t_d[:, t2, :, :])
            xr16 = pxt.tile([128, KC, 256], FP8, tag="xr16", name=f"xr16_{t2}")
            nc.sync.dma_start(xr16[:], xr16t_d[:, t2, :, :])
            msk = pxt.tile([1, 256], FP8, tag="msk", name=f"msk{t2}")
            nc.sync.dma_start(msk[:], maskv8[0:1, 256 * t2 : 256 * (t2 + 1)])
            xt_by_t2[t2] = (xt8, xr16, msk)

        def finalize(pi):
            """Emit deferred paired S-matmuls for pair pi (+ tile epilogue)."""
            g2_, spv_, prt_, first, last, q_ = state.pop(pi)
            # prt spans 2 PSUM banks (4 chunks each); only the first chunk
            # per bank may set start.
            for k in range(KC):
                nc.tensor.matmul(
                    prt_[:, k, :],
                    g2_[:, :, 128 * k : 128 * (k + 1)],
                    spv_,
                    start=(first and k % 4 == 0),
                    stop=last,
                    perf_mode=DR,
                )
            if last:
                t2_, half_ = q_ // 2, q_ % 2
                rmt_ = rmt_by_t2[t2_]
                nc.vector.tensor_copy(
                    rmt_[:, :KC, 128 * half_ : 128 * (half_ + 1)], prt_[:]
                )
                if half_ == 1:
                    pending_mlp2.append((t2_, rmt_))

        def mlp2(t2, rmt):
            xt8, xr16, _msk = xt_by_t2.pop(t2)

            o1 = po1.tile([128, KC, 256], BF16, tag="o1", name=f"o1_{t2}")
            o18s4 = po1.tile([128, KC, 256], FP8, tag="o18s4", name=f"o18s4_{t2}")
            o18d16 = po1.tile([128, KC, 256], FP8, tag="o18d16", name=f"o18d16_{t2}")
            rr4 = po1.tile([128, KC, 256], FP8, tag="rr4", name=f"rr4_{t2}")
            for m in range(KC):
                pb = psMm2.tile([128, 256], F32, tag="pb", name=f"pa{t2}_{m}")
                ms = slice(128 * m, 128 * (m + 1))
                for k4 in range(K4):
                    ks = slice(2 * k4, 2 * k4 + 2)
                    nc.tensor.matmul(
                        pb[:], b1s_sb[:, ks, ms], xt8[:, ks, :],
                        start=(k4 == 0), stop=False, perf_mode=DR,
                    )
                for k4 in range(K4):
                    ks = slice(2 * k4, 2 * k4 + 2)
                    nc.tensor.matmul(
                        pb[:], b1p_sb[:, ks, ms], xr16[:, ks, :],
                        start=False, stop=False, perf_mode=DR,
                    )
                for k4 in range(K4):
                    ks = slice(2 * k4, 2 * k4 + 2)
                    nc.tensor.matmul(
                        pb[:], b1r_sb[:, ks, ms], xt8[:, ks, :],
                        start=False, stop=False, perf_mode=DR,
                    )
                # W3 split: the even tile's rm half (cols 0:128) was copied a
                # whole tile earlier; the odd half just landed, so do it last.
                # The 5th chunk-pair carries mask (x) u16 — the folded u-term.
                for k4 in range(K4 + 1):
                    ks = slice(2 * k4, 2 * k4 + 2)
                    nc.tensor.matmul(
                        pb[:, 0:128], w3s_sb[:, ks, ms], rmt[:, ks, 0:128],
                        start=False, stop=False, perf_mode=DR,
                    )
                for k4 in range(K4 + 1):
                    ks = slice(2 * k4, 2 * k4 + 2)
                    nc.tensor.matmul(
                        pb[:, 128:256], w3s_sb[:, ks, ms], rmt[:, ks, 128:256],
                        start=False, stop=(k4 == K4), perf_mode=DR,
                    )
                nc.scalar.activation(o1[:, m, :], pb[:], AF.Relu,
                                     bias=b2a_sb[:, m : m + 1])
                # fp8 forms for the o2 residual matmuls: o18s4 = fp8(4*o1s16)
                # (= 64*o1_q), o18d16 = fp8(o1s16/16) (= o1_q), rr4 =
                # 4*o1s16 - o18s4 (= 64*(o1 - o1_q))
                nc.scalar.activation(o18s4[:, m, :], o1[:, m, :], AF.Identity,
                                     scale=4.0)
                nc.scalar.activation(o18d16[:, m, :], o1[:, m, :], AF.Identity,
                                     scale=1.0 / 16)
                nc.vector.scalar_tensor_tensor(
                    out=rr4[:, m, :], in0=o1[:, m, :], scalar=4.0,
                    in1=o18s4[:, m, :], op0=mybir.AluOpType.mult,
                    op1=mybir.AluOpType.subtract,
                )

            o2 = po1.tile([128, KC, 256], BF16, tag="o2", name=f"o2_{t2}",
                          bufs=3)
            for m in range(KC):
                pb = psMm2.tile([128, 256], F32, tag="pb", name=f"pb{t2}_{m}")
                ms = slice(128 * m, 128 * (m + 1))
                for k4 in range(K4):
                    ks = slice(2 * k4, 2 * k4 + 2)
                    nc.tensor.matmul(
                        pb[:], w2b8_sb[:, ks, ms], o18s4[:, ks, :],
                        start=(k4 == 0), stop=False, perf_mode=DR,
                    )
                for k4 in range(K4):
                    ks = slice(2 * k4, 2 * k4 + 2)
                    nc.tensor.matmul(
                        pb[:], w2b8_sb[:, ks, ms], rr4[:, ks, :],
                        start=False, stop=False, perf_mode=DR,
                    )
                for k4 in range(K4):
                    ks = slice(2 * k4, 2 * k4 + 2)
                    nc.tensor.matmul(
                        pb[:], w2br_sb[:, ks, ms], o18d16[:, ks, :],
                        start=False, stop=(k4 == K4 - 1), perf_mode=DR,
                    )
                nc.scalar.activation(o2[:, m, :], pb[:], AF.Identity,
                                     bias=b2b_sb[:, m : m + 1], scale=1.0 / 64)
                if t2 == NT // 2 - 1:
                    nc.sync.dma_start(
                        out_myT[:, m : m + 1, 256 * t2 : 256 * (t2 + 1)],
                        o2[:, m : m + 1, :],
                    )
            if t2 != NT // 2 - 1:
                nc.gpsimd.dma_start(out_myT[:, :, 256 * t2 : 256 * (t2 + 1)], o2[:])

        for pi, (q, js) in enumerate(pairs):
            t2, half = q // 2, q % 2
            first_in_q = js[0] == sum(pattern[:q])
            if half == 0 and first_in_q:
                rmt = prm.tile(
                    [128, KC + 2, 256], FP8, tag="rm", name=f"rm{t2}"
                )
                rmt_by_t2[t2] = rmt
                # chunk KC carries (mask x u16) for the folded u-term; KC+1
                # is a zero pad (both sides zero; fp8 garbage x 0 = NaN)
                nc.gpsimd.memset(rmt[:, KC : KC + 2, :], 0.0)
                prefetch_xt(t2)
                msk8 = xt_by_t2[t2][2]
                nc.gpsimd.tensor_copy(rmt[0:1, KC, :], msk8[:])
            if first_in_q:
                prt_by_q[q] = psPr.tile([128, KC, 128], F32, tag="pr", name=f"prt{q}")
            prt = prt_by_q[q]

            psb, pcol = divmod(pi, 4)
            if pcol == 0:
                hi = min(4, NPAIR - 4 * psb)
                sp_sb = ssp.tile([128, 4, 2, 128], FP8, tag="sp", name=f"sp{psb}")
                nc.sync.dma_start(
                    sp_sb[:, :hi, :, :], sp_d[:, 4 * psb : 4 * psb + hi, :, :]
                )
            spv = sp_sb[:, pcol, :, :]

            g2 = pg2.tile([128, 2, D], FP8, tag="g2", name=f"g2_{pi}")
            if len(js) == 1:
                # half pair: unused g half could hold fp8 NaN garbage; its sp
                # half is 0 but NaN*0 = NaN, so zero it
                nc.gpsimd.memset(g2[:, 1, :], 0.0)
            for gi, jj in enumerate(js):
                sb, col = divmod(jj, 4)
                if col == 0:
                    load_eat(sb + 2)   # 2-superblock lookahead (bufs=3)
                    eat = eat_tiles.pop(sb)
                xwg = pgat.tile([128, D], FP8, tag="xwg", name=f"xwg{jj}")
                nc.gpsimd.indirect_dma_start(
                    out=xwg[:],
                    out_offset=None,
                    in_=xw_dram[:],
                    in_offset=bass.IndirectOffsetOnAxis(
                        ap=srcidx_sb[:, jj : jj + 1], axis=0
                    ),
                )
                ph = psBig.tile([128, KC, 128], F32, tag="big", name=f"ph{jj}")
                for h in range(2):
                    for k4 in range(K4):
                        nc.tensor.matmul(
                            ph[:, 4 * h : 4 * (h + 1), :],
                            eat[:, 2 * k4 : 2 * k4 + 2,
                                128 * col : 128 * (col + 1)],
                            w1a_sb[:, KC + 2 * k4 : KC + 2 * k4 + 2,
                                   512 * h : 512 * (h + 1)],
                            start=(k4 == 0),
                            stop=(k4 == K4 - 1),
                            perf_mode=DR,
                        )
                gt = pg.tile([128, D], BF16, tag="gt", name=f"gt{jj}")
                nc.vector.tensor_tensor(out=gt[:], in0=ph[:], in1=xwg[:],
                                        op=mybir.AluOpType.add)
                nc.scalar.activation(g2[:, gi, :], gt[:], AF.Relu)

            is_first = js[0] == sum(pattern[:q])
            is_last = js[-1] == sum(pattern[: q + 1]) - 1
            target = pi - (DEFER0 if pi < DEFER0 + 5 else DEFER)
            while next_fin <= target:
                finalize(next_fin)
                next_fin += 1
            while pending_mlp2:
                mlp2(*pending_mlp2.pop(0))
            state[pi] = (g2, spv, prt, is_first, is_last, q)
        while next_fin < NPAIR:
            finalize(next_fin)
            next_fin += 1
            while pending_mlp2:
                mlp2(*pending_mlp2.pop(0))

    nc.compile()
    return nc


def _get_program(NG, GU, pattern):
    key = (NG, GU, tuple(pattern))
    if key not in _PROGRAM_CACHE:
        _PROGRAM_CACHE[key] = _build_program(NG, GU, tuple(pattern))
    return _PROGRAM_CACHE[key]


def _pack_core(deg):
    """Pack NPC nodes (weights deg) into NT bins, <=128 nodes each,
    minimizing sum(ceil(load/128)). Returns list of (nodes, load)."""
    order = np.argsort(-deg, kind="stable")
    nodes = [[] for _ in range(NT)]
    load = np.zeros(NT, np.int64)
    cnt = np.zeros(NT, np.int64)
    for n in order:
        # LPT with node cap
        cand = [b for b in range(NT) if cnt[b] < 128]
        b = min(cand, key=lambda b: (load[b], cnt[b]))
        nodes[b].append(n)
        load[b] += deg[n]
        cnt[b] += 1
    # refinement: reduce sum(ceil(load/128)) by moving small nodes out of
    # bins that spill just over a multiple of 128
    for _ in range(200):
        ceil = -(-load // 128)
        improved = False
        spill_key = np.where(
            (load > 0) & (load % 128 != 0), (load - 1) % 128 + 1, 10**9
        )
        for a in np.argsort(spill_key):
            if load[a] == 0 or (load[a] % 128) == 0:
                continue
            spill = load[a] - 128 * (ceil[a] - 1)
            # try to move small nodes (total <= spill) from a to other bins
            small = sorted((deg[n], n) for n in nodes[a] if deg[n] > 0)
            moved = []
            need = spill
            for d, n in small:
                if d > need:
                    break
                tgt = None
                for b in range(NT):
                    if b == a or cnt[b] >= 128:
                        continue
                    if -(-(load[b] + d) // 128) == ceil[b]:
                        tgt = b
                        break
                if tgt is None:
                    continue
                nodes[a].remove(n)
                nodes[tgt].append(n)
                load[a] -= d
                load[tgt] += d
                cnt[a] -= 1
                cnt[tgt] += 1
                moved.append(n)
                need -= d
                if need <= 0:
                    break
            if need <= 0 and moved:
                improved = True
                break
        if not improved:
            break
    return [(nodes[b], int(load[b])) for b in range(NT)]


def _chunk_major(mat):
    """[D, n] f32/other -> [128, KC, n] fp8 chunk-major: out[p, c, j] =
    mat[128c+p, j]."""
    Dd, n = mat.shape
    assert Dd == D
    return np.ascontiguousarray(
        mat.reshape(KC, 128, n).transpose(1, 0, 2)
    )


def _make_in_maps(x, edge_index, edge_attr, W1a, b1a, W1b, b1b, W2a, b2a, W2b, b2b):
    x = np.ascontiguousarray(np.asarray(x, np.float32))
    edge_attr = np.ascontiguousarray(np.asarray(edge_attr, np.float32))
    ei = np.asarray(edge_index)
    row, col = ei[0].astype(np.int64), ei[1].astype(np.int64)

    perm = np.argsort(col, kind="stable")
    col_s, row_s = col[perm], row[perm]
    core_bounds = np.searchsorted(col_s, NPC * np.arange(C + 1))
    counts = np.bincount(col, minlength=N)

    # ---- pack nodes per core; derive the shared group pattern ----
    packs, uniqs = [], []
    for c in range(C):
        deg = counts[NPC * c : NPC * (c + 1)]
        bins = _pack_core(deg)
        bins.sort(key=lambda bl: -bl[1])
        packs.append(bins)
        s0, e0 = core_bounds[c], core_bounds[c + 1]
        uniqs.append(np.unique(row_s[s0:e0]))
    pattern = tuple(
        int(max(-(-packs[c][q][1] // 128) for c in range(C))) for q in range(NT)
    )
    pattern = tuple(max(p, 1) for p in pattern)
    NG = sum(pattern)
    GU = max(1, -(-max(len(u) for u in uniqs) // 128))
    UP, S = GU * 128, NG * 128
    pairs = _pair_schedule(pattern)
    NPAIR = len(pairs)
    # map group jj -> (pair index, half within pair)
    pair_idx_of_jj = np.zeros(NG, np.int64)
    half_of_jj = np.zeros(NG, np.int64)
    for pi, (q, js) in enumerate(pairs):
        for gi, jj in enumerate(js):
            pair_idx_of_jj[jj] = pi
            half_of_jj[jj] = gi

    # ---- fold weights (float64 for accuracy) ----
    B1 = np.asarray(W2a, np.float64)[:D].astype(np.float32)
    B2 = np.asarray(W2a, np.float64)[D:]
    W3 = (np.asarray(W1b, np.float64) @ B2).astype(np.float32)
    u_vec = (np.asarray(b1b, np.float64) @ B2).astype(np.float32)

    # fp8 residual split of B1 (x@B1 = x8@B1_8s16/16 + xr16@B1_8/16 + x8@B1_r16/16)
    B1_8 = B1.astype(E4M3)
    B1_8s16 = (16.0 * B1_8.astype(np.float32)).astype(E4M3)
    B1_r16 = (16.0 * (B1 - B1_8.astype(np.float32))).astype(E4M3)
    W3_s16 = (16.0 * W3).astype(E4M3)

    x8 = x.astype(E4M3)
    xr16 = (16.0 * (x - x8.astype(np.float32))).astype(E4M3)
    ea8 = edge_attr.astype(E4M3)

    # W3 chunk-major with 2 extra chunks: [p, KC, n] = u16[n] at p==0 (the
    # folded u-term, contracted against the mask row in rm chunk KC)
    W2bf = np.asarray(W2b, np.float32)
    W2b8 = W2bf.astype(E4M3)
    w2b8_cm = _chunk_major(W2b8)
    w2br_cm = _chunk_major((64.0 * (W2bf - W2b8.astype(np.float32))).astype(E4M3))

    w3s_ext = np.zeros((128, KC + 2, D), E4M3)
    w3s_ext[:, :KC, :] = _chunk_major(W3_s16)
    w3s_ext[0, KC, :] = (16.0 * u_vec).astype(E4M3)

    # w1a is [2D, D]: build [128, 2*KC, D]
    w1a8 = np.asarray(W1a, np.float32).astype(E4M3)
    w1a3 = np.ascontiguousarray(
        w1a8.reshape(2 * KC, 128, D).transpose(1, 0, 2)
    )
    b1a2 = np.zeros((1, 2, D), E4M3)
    b1a2[0, 0, :] = np.asarray(b1a, np.float32).astype(E4M3)
    ones2 = np.ones((1, 2, 128), E4M3)

    in_maps = []
    orders = []
    for c in range(C):
        s0 = core_bounds[c]
        lo = NPC * c
        bins = packs[c]
        uniq = uniqs[c]
        deg = counts[lo : lo + NPC]
        starts = np.zeros(NPC + 1, np.int64)
        np.cumsum(deg, out=starts[1:])

        src_l = np.zeros((128, NG), np.int32)     # local xw row per slot
        sp = np.zeros((128, NPAIR, 2, 128), E4M3)  # paired selection matrices
        ea_sel = np.full(S, -1, np.int64)         # edge_attr row per slot
        order = np.full(NP, -1, np.int64)         # packed node order

        goff = 0
        for q in range(NT):
            bnodes, load = bins[q]
            Gq = pattern[q]
            pos = 0
            for p, n in enumerate(bnodes):
                order[128 * q + p] = n
                d = int(deg[n])
                if d == 0:
                    continue
                ids = np.arange(starts[n], starts[n + 1], dtype=np.int64)
                sl = goff * 128 + pos + np.arange(d)
                gidx_, ridx_ = sl // 128, sl % 128
                src_l[ridx_, gidx_] = np.searchsorted(uniq, row_s[s0 + ids])
                sp[ridx_, pair_idx_of_jj[gidx_], half_of_jj[gidx_], p] = E4M3(
                    1.0 / d
                )
                ea_sel[sl] = perm[s0 + ids]
                pos += d
            assert pos <= 128 * Gq, (c, q, pos, Gq)
            goff += Gq
        assert goff == NG

        # chunk-major transposed, slot-ordered edge features (fp8)
        eaT = np.zeros((D, S), E4M3)
        nz = ea_sel >= 0
        eaT[:, nz] = ea8[ea_sel[nz]].T
        eat3 = _chunk_major(eaT)
        # chunk-major transposed unique-source features (fp8)
        xtuT = np.zeros((D, UP), E4M3)
        xtuT[:, : len(uniq)] = x8[uniq].T
        xtu3 = _chunk_major(xtuT)

        ordc = np.maximum(order, 0)
        valid = order >= 0
        cnt_c = np.where(valid, deg[ordc], 0)
        mask_c = ((cnt_c > 0) & valid).astype(E4M3)
        x8_c = np.where(valid[:, None], x8[lo + ordc], E4M3(0.0))
        xr16_c = np.where(valid[:, None], xr16[lo + ordc], E4M3(0.0))

        in_maps.append(
            {
                "xtu_d": xtu3,
                "eat_d": eat3,
                "sp_d": sp,
                "srcidx": src_l,
                "x8t_d": np.ascontiguousarray(_chunk_major(np.ascontiguousarray(x8_c.T)).reshape(128, KC, NT // 2, 256).transpose(0, 2, 1, 3)),
                "xr16t_d": np.ascontiguousarray(_chunk_major(np.ascontiguousarray(xr16_c.T)).reshape(128, KC, NT // 2, 256).transpose(0, 2, 1, 3)),
                "maskv8": mask_c.reshape(1, NP),
                "ones2_d": ones2,
                "w1a_d": w1a3,
                "b1s_d": _chunk_major(B1_8s16.astype(E4M3)),
                "b1p_d": _chunk_major(B1_8),
                "b1r_d": _chunk_major(B1_r16),
                "w3s_d": w3s_ext,
                "w2b8_d": w2b8_cm,
                "w2br_d": w2br_cm,
                "b1a2_d": b1a2,
                "b2a_d": 16.0 * np.asarray(b2a, np.float32).reshape(KC, 128).T,
                "b2b_d": np.asarray(b2b, np.float32).reshape(KC, 128).T.copy(),
            }
        )
        orders.append(order)
    return (NG, GU, pattern), in_maps, orders


def kernel(x, edge_index, edge_attr, W1a, b1a, W1b, b1b, W2a, b2a, W2b, b2b):
    args, in_maps, orders = _make_in_maps(
        x, edge_index, edge_attr, W1a, b1a, W1b, b1b, W2a, b2a, W2b, b2b
    )
    nc = _get_program(*args)
    res = run_bass_kernel_spmd(nc, in_maps, core_ids=list(range(C)))
    out = np.empty((N, D), np.float32)
    for c in range(C):
        o = np.asarray(res.results[c]["out_myT"]).astype(np.float32)
        o = o.transpose(2, 1, 0).reshape(NP, D)  # [n, 128c+p] from [p, c, n]
        order = orders[c]
        valid = order >= 0
        out[NPC * c + order[valid]] = o[valid]
    return np.ascontiguousarray(out)


# revision 82
# speedup vs baseline: 1.0016x; 1.0016x over previous
"""GNN NodeModel kernel for 8 Trainium2 NeuronCores (Bass/Tile) — fp8 V5.

Design (per core, nodes sharded 2500/core, edges sorted by destination).
All big matmuls run in fp8-e4m3 DoubleRow perf mode (256-deep contraction,
0.5 cycles/row — 4x the bf16 instruction cost per unit work); f32 PSUM.

  Phase 1 (xw): dedup sources — xw = x8[u] @ W1a_top8 + b1a8 for the core's
    ~8000 unique source nodes (host supplies x8[u] chunk-major transposed
    fp8), bias added via a K=1 DoubleRow ones-matmul; fp8 rows to a DRAM
    scratch. pw rotates over 3 PSUM buffers (psBig x2 + psPr, idle until
    phase 2); the PSUM->SBUF casts split per half across DVE+ACT so the
    copy chain hides inside the next ug's PE window.
  Phase 2 (fused edge + segment): nodes packed into 20 tiles of <=128
    nodes; each tile owns G_q 128-edge groups, processed in PAIRS so the
    segment matmuls can also run DoubleRow (256-edge contraction).
    Per group: gather xw rows (indirect DMA, fp8); ph = ea8 @ W1a_bot8
    (4 DR matmuls x 2 halves); gt = ph + xw_g (DVE); g = relu(gt) -> fp8
    half of the pair tile. Per pair: prT[:,k,:] += [gA|gB].T @ [spA|spB]
    (DR, deferred 3 pairs so the gather/add/relu chain stays off the PE
    critical path), where sp holds fp8(1/deg) — the accumulated result is
    the mean, feature-major, ready for MLP2.
  Phase 3 (mlp2, interleaved per 2 node tiles): folded node MLP2:
      o1 = relu(x8@B1_8s16 + xr16@B1_8 + x8@B1_r16 + rm8@(16W3)_8
                + (mask x 16u, as a 5th DR chunk-pair) + 16 b2a)
      out = o1 @ W2b + b2b     [x@B1 and o1@W2b both exact via 3-term
                                fp8 residuals: W2b = w2b8 + w2br64/64 and
                                o1 = (o18s4 + rr4)/64, PSUM at 64x]
    W3 = W1b@W2a_bot, u = b1b@W2a_bot (host-folded). Output written
    chunk-major transposed bf16, unpacked on host.

  Queues: SP = input streams (xu/eat/sp/xt superblocks, xw writes);
  Pool = weight loads (gated mid-phase-1 off xw_dram rows to dodge the
  serial DMA_ENGINES device) + indirect gathers + out writes; ACT/DVE =
  casts, adds, relus. TimelineSim-modeled 332.4us/core (baseline 810.6).
"""

import sys

sys.path.insert(0, "/opt/trn_rl_repo")

from contextlib import ExitStack

import numpy as np
import ml_dtypes

import concourse.bass as bass
import concourse.tile as tile
from concourse import bacc, mybir
from concourse.bass_utils import run_bass_kernel_spmd

N = 20000
E = 80000
D = 1024
C = 8
NPC = N // C      # 2500 nodes per core
NT = 20           # node tiles per core
NP = NT * 128     # 2560 padded node slots
KC = D // 128     # 8 feature chunks of 128
K4 = D // 256     # 4 feature chunks of 256 (DoubleRow)
F32 = mybir.dt.float32
BF16 = mybir.dt.bfloat16
FP8 = mybir.dt.float8e4
I32 = mybir.dt.int32
BF = ml_dtypes.bfloat16
E4M3 = ml_dtypes.float8_e4m3
DR = mybir.MatmulPerfMode.DoubleRow

AF = mybir.ActivationFunctionType

_PROGRAM_CACHE = {}
DEFAULT_PROGRAM_ARGS = (80, 63, (4,) * 20)


def _pair_schedule(pattern):
    """Groups per node tile paired up: list of (q, [jjA] or [jjA, jjB])."""
    pairs = []
    jj = 0
    for q in range(NT):
        js = list(range(jj, jj + pattern[q]))
        jj += pattern[q]
        for i in range(0, len(js), 2):
            pairs.append((q, js[i : i + 2]))
    return pairs


def _build_program(NG, GU, pattern):
    """NG: total 128-edge groups/core (sum of pattern); GU: 128-row groups of
    unique source nodes; pattern: groups per node tile (len NT)."""
    assert sum(pattern) == NG and len(pattern) == NT
    UP = GU * 128
    S = NG * 128
    pairs = _pair_schedule(pattern)
    NPAIR = len(pairs)

    nc = bacc.Bacc("TRN2", target_bir_lowering=False, debug=False, num_devices=C)

    # ---- DRAM I/O (fp8 tensors are chunk-major: [p, c, n] = M[128c+p, n]) --
    xtu_d = nc.dram_tensor("xtu_d", [128, KC, UP], FP8, kind="ExternalInput").ap()
    eat_d = nc.dram_tensor("eat_d", [128, KC, S], FP8, kind="ExternalInput").ap()
    sp_d = nc.dram_tensor("sp_d", [128, NPAIR, 2, 128], FP8, kind="ExternalInput").ap()
    srcidx = nc.dram_tensor("srcidx", [128, NG], I32, kind="ExternalInput").ap()
    x8t_d = nc.dram_tensor("x8t_d", [128, NT // 2, KC, 256], FP8, kind="ExternalInput").ap()
    xr16t_d = nc.dram_tensor("xr16t_d", [128, NT // 2, KC, 256], FP8, kind="ExternalInput").ap()
    maskv8 = nc.dram_tensor("maskv8", [1, NP], FP8, kind="ExternalInput").ap()
    ones2_d = nc.dram_tensor("ones2_d", [1, 2, 128], FP8, kind="ExternalInput").ap()
    w1a_d = nc.dram_tensor("w1a_d", [128, 2 * KC, D], FP8, kind="ExternalInput").ap()
    b1s_d = nc.dram_tensor("b1s_d", [128, KC, D], FP8, kind="ExternalInput").ap()
    b1p_d = nc.dram_tensor("b1p_d", [128, KC, D], FP8, kind="ExternalInput").ap()
    b1r_d = nc.dram_tensor("b1r_d", [128, KC, D], FP8, kind="ExternalInput").ap()
    w3s_d = nc.dram_tensor("w3s_d", [128, KC + 2, D], FP8, kind="ExternalInput").ap()
    w2b8_d = nc.dram_tensor("w2b8_d", [128, KC, D], FP8, kind="ExternalInput").ap()
    w2br_d = nc.dram_tensor("w2br_d", [128, KC, D], FP8, kind="ExternalInput").ap()
    b1a2_d = nc.dram_tensor("b1a2_d", [1, 2, D], FP8, kind="ExternalInput").ap()
    b2a_d = nc.dram_tensor("b2a_d", [128, KC], F32, kind="ExternalInput").ap()
    b2b_d = nc.dram_tensor("b2b_d", [128, KC], F32, kind="ExternalInput").ap()
    out_myT = nc.dram_tensor("out_myT", [128, KC, NP], BF16, kind="ExternalOutput").ap()
    xw_dram = nc.dram_tensor("xw_scratch", [UP, D], FP8).ap()

    NSB_U = (GU + 3) // 4   # xTu superblocks of 512 cols
    NSB_P = (NPAIR + 3) // 4  # sp superblocks of 4 pairs

    with tile.TileContext(nc) as tc, ExitStack() as ctx:
        cpool = ctx.enter_context(tc.tile_pool(name="consts", bufs=1))
        wpool = ctx.enter_context(tc.tile_pool(name="weights", bufs=1))
        sxu = ctx.enter_context(tc.tile_pool(name="sxu", bufs=4))
        sea = ctx.enter_context(tc.tile_pool(name="sea", bufs=3))
        ssp = ctx.enter_context(tc.tile_pool(name="ssp", bufs=2))
        pgat = ctx.enter_context(tc.tile_pool(name="pgat", bufs=12))
        pg = ctx.enter_context(tc.tile_pool(name="pg", bufs=4))
        pg2 = ctx.enter_context(tc.tile_pool(name="pg2", bufs=6))
        pxw = ctx.enter_context(tc.tile_pool(name="pxw", bufs=24))
        prm = ctx.enter_context(tc.tile_pool(name="prm", bufs=2))
        pxt = ctx.enter_context(tc.tile_pool(name="pxt", bufs=2))
        po1 = ctx.enter_context(tc.tile_pool(name="po1", bufs=2))
        psBig = ctx.enter_context(tc.tile_pool(name="psBig", bufs=2, space="PSUM"))
        psPr = ctx.enter_context(tc.tile_pool(name="psPr", bufs=1, space="PSUM"))
        psMm2 = ctx.enter_context(tc.tile_pool(name="psMm2", bufs=2, space="PSUM"))

        # ---- phase-1-critical loads first, in consumption order: the
        # k4-chunk pairs arrive as the PE's contraction loop needs them;
        # ones2/b1a2 only matter at the 5th matmul (bias)
        w1a_sb = wpool.tile([128, 2 * KC, D], FP8, tag="w1a")
        hi0 = min(512, UP)
        xu = sxu.tile([128, KC, 512], FP8, tag="xu", name="xu0")
        # spread the startup-critical loads across queues so the issue
        # overhead parallelizes (transfers still serialize on DMA_ENGINES)
        ones2_sb = cpool.tile([1, 2, 128], FP8, tag="ones2")
        nc.scalar.dma_start(ones2_sb[:], ones2_d[:])
        b1a2_sb = cpool.tile([1, 2, D], FP8, tag="b1a2")
        nc.scalar.dma_start(b1a2_sb[:], b1a2_d[:])
        nc.sync.dma_start(w1a_sb[:, 0:2, :], w1a_d[:, 0:2, :])
        nc.sync.dma_start(xu[:, 0:2, :hi0], xtu_d[:, 0:2, :hi0])
        nc.scalar.dma_start(w1a_sb[:, 2:4, :], w1a_d[:, 2:4, :])
        nc.gpsimd.dma_start(w1a_sb[:, 4:6, :], w1a_d[:, 4:6, :])
        nc.sync.dma_start(w1a_sb[:, 6:KC, :], w1a_d[:, 6:KC, :])
        nc.gpsimd.dma_start(xu[:, 2:4, :hi0], xtu_d[:, 2:4, :hi0])
        nc.sync.dma_start(xu[:, 4:KC, :hi0], xtu_d[:, 4:KC, :hi0])
        xu_tiles = {0: xu}

        def load_xu(sb):
            if sb >= NSB_U or sb in xu_tiles:
                return
            hi = min(512, UP - 512 * sb)
            t = sxu.tile([128, KC, 512], FP8, tag="xu", name=f"xu{sb}")
            nc.sync.dma_start(t[:, :, :hi], xtu_d[:, :, 512 * sb : 512 * sb + hi])
            xu_tiles[sb] = t

        load_xu(1)
        load_xu(2)
        NSB_E = (NG + 3) // 4
        eat_tiles = {}

        def load_eat(sb):
            if sb >= NSB_E or sb in eat_tiles:
                return
            hi = min(512, S - 512 * sb)
            t = sea.tile([128, KC, 512], FP8, tag="ea", name=f"ea{sb}")
            nc.sync.dma_start(t[:, :, :hi], eat_d[:, :, 512 * sb : 512 * sb + hi])
            eat_tiles[sb] = t
        # srcidx early: first gather fires right at phase-2 start
        srcidx_sb = cpool.tile([128, NG], I32, tag="srcidx")
        nc.sync.dma_start(srcidx_sb[:], srcidx[:])

        # ---- phase 2/3 weights: Pool queue, gated behind a dummy copy that
        # depends on ug 6's output so their transfers don't contend with the
        # startup-critical xu/w1a loads on the (exclusive) DMA_ENGINES device
        b2a_sb = cpool.tile([128, KC], F32, tag="b2a")
        b2b_sb = cpool.tile([128, KC], F32, tag="b2b")
        b1s_sb = wpool.tile([128, KC, D], FP8, tag="b1s")
        b1p_sb = wpool.tile([128, KC, D], FP8, tag="b1p")
        b1r_sb = wpool.tile([128, KC, D], FP8, tag="b1r")
        w3s_sb = wpool.tile([128, KC + 2, D], FP8, tag="w3s")
        w2b8_sb = wpool.tile([128, KC, D], FP8, tag="w2b8")
        w2br_sb = wpool.tile([128, KC, D], FP8, tag="w2br")
        zeros_sb = cpool.tile([128, 256], BF16, tag="zeros")
        nc.vector.memset(zeros_sb[:], 0.0)

        wgate = cpool.tile([1, 64], FP8, tag="wgate")

        wgate2 = cpool.tile([1, 64], FP8, tag="wgate2")

        def emit_weight_loads(gate_ug):
            # phase-2-start-critical only (w1a bottom + small consts).
            # gate: Pool DMA reading the xw row written at gate_ug — releases
            # as soon as that SP write lands
            nc.gpsimd.dma_start(wgate[:], xw_dram[128 * gate_ug : 128 * gate_ug + 1, 0:64])
            nc.gpsimd.dma_start(b2a_sb[:], b2a_d[:])
            nc.gpsimd.dma_start(b2b_sb[:], b2b_d[:])
            for c2 in range(0, KC, 2):
                nc.gpsimd.dma_start(w1a_sb[:, KC + c2 : KC + c2 + 2, :],
                                    w1a_d[:, KC + c2 : KC + c2 + 2, :])

        def emit_mlp2_weight_loads(gate_ug):
            # mlp2-only weights: land in phase 1's write-only final stretch
            nc.gpsimd.dma_start(wgate2[:], xw_dram[128 * gate_ug : 128 * gate_ug + 1, 0:64])
            for c2 in range(0, KC, 2):
                cs = slice(c2, c2 + 2)
                nc.gpsimd.dma_start(b1s_sb[:, cs, :], b1s_d[:, cs, :])
                nc.gpsimd.dma_start(b1p_sb[:, cs, :], b1p_d[:, cs, :])
                nc.gpsimd.dma_start(b1r_sb[:, cs, :], b1r_d[:, cs, :])
                nc.gpsimd.dma_start(w3s_sb[:, cs, :], w3s_d[:, cs, :])
                nc.gpsimd.dma_start(w2b8_sb[:, cs, :], w2b8_d[:, cs, :])
                nc.gpsimd.dma_start(w2br_sb[:, cs, :], w2br_d[:, cs, :])
            nc.gpsimd.dma_start(w3s_sb[:, KC : KC + 2, :],
                                w3s_d[:, KC : KC + 2, :])

        # ================= Phase 1: xw = x8_u @ W1a_top8 + b1a ==============
        for ug in range(GU):
            sb, col = divmod(ug, 4)
            if col == 0:
                load_xu(sb + 3)   # 3-superblock lookahead (bufs=4)
                xu = xu_tiles.pop(sb)
            # rotate over 3 PSUM buffers: psBig's two + psPr's one (psPr is
            # idle until phase 2 and its 'pr' tag has the same [128,D] f32
            # shape, so the static footprint is unchanged)
            if ug % 3 < 2:
                pw = psBig.tile([128, KC, 128], F32, tag="big", name=f"pw{ug}")
            else:
                pw = psPr.tile([128, KC, 128], F32, tag="pr", name=f"pw{ug}")
            for h in range(2):
                for k4 in range(K4):
                    nc.tensor.matmul(
                        pw[:, 4 * h : 4 * (h + 1), :],
                        xu[:, 2 * k4 : 2 * k4 + 2, 128 * col : 128 * (col + 1)],
                        w1a_sb[:, 2 * k4 : 2 * k4 + 2, 512 * h : 512 * (h + 1)],
                        start=(k4 == 0),
                        stop=False,
                        perf_mode=DR,
                    )
                nc.tensor.matmul(
                    pw[:, 4 * h : 4 * (h + 1), :],
                    ones2_sb[0:1, :, :],
                    b1a2_sb[0:1, :, 512 * h : 512 * (h + 1)],
                    start=False,
                    stop=True,
                    perf_mode=DR,
                )
            xw_sb = pxw.tile([128, D], FP8, tag="xw", name=f"xwsb{ug}")
            # quarter-split PSUM->SBUF copies on ACT+DVE, each starting as
            # soon as its PSUM half stops, so the ~0.55us sem latency plus
            # copy time hides inside the next ug's PE window
            nc.vector.tensor_copy(xw_sb[:, 0:512], pw[:, 0:4, :])
            nc.scalar.activation(xw_sb[:, 512:D], pw[:, 4:8, :], AF.Identity)
            nc.sync.dma_start(xw_dram[128 * ug : 128 * (ug + 1), :], xw_sb[:])
            if ug == min(3, GU - 1):
                emit_weight_loads(max(0, ug - 1))
            if ug == max(min(12, GU - 1), GU - 26):
                emit_mlp2_weight_loads(max(0, ug - 1))
            if ug == max(0, GU - 16):
                # preload first edge superblocks late in phase 1 so they're
                # resident the moment phase 2 starts (SP queue is in-order)
                load_eat(0)
                load_eat(1)

        # ============ Phase 2: fused edge MLP1 + paired segment means ======
        DEFER = 3           # pairs of S-matmul deferral (relu-chain cover)
        DEFER0 = 3          # deeper deferral at the phase boundary (xw-write
                            # -> gather -> add -> relu chain is ~7us)
        next_fin = 0        # next pair to finalize
        state = {}          # pi -> (g2, spv, prt, first, last, q)
        rmt_by_t2 = {}
        prt_by_q = {}
        pending_mlp2 = []
        sp_sb = None
        xt_by_t2 = {}

        def prefetch_xt(t2):
            xt8 = pxt.tile([128, KC, 256], FP8, tag="xt8", name=f"xt8_{t2}")
            nc.sync.dma_start(xt8[:], x8t_d[:, t2, :, :])
            xr16 = pxt.tile([128, KC, 256], FP8, tag="xr16", name=f"xr16_{t2}")
            nc.sync.dma_start(xr16[:], xr16t_d[:, t2, :, :])
            msk = pxt.tile([1, 256], FP8, tag="msk", name=f"msk{t2}")
            nc.sync.dma_start(msk[:], maskv8[0:1, 256 * t2 : 256 * (t2 + 1)])
            xt_by_t2[t2] = (xt8, xr16, msk)

        def finalize(pi):
            """Emit deferred paired S-matmuls for pair pi (+ tile epilogue)."""
            g2_, spv_, prt_, first, last, q_ = state.pop(pi)
            # prt spans 2 PSUM banks (4 chunks each); only the first chunk
            # per bank may set start.
            for k in range(KC):
                nc.tensor.matmul(
                    prt_[:, k, :],
                    g2_[:, :, 128 * k : 128 * (k + 1)],
                    spv_,
                    start=(first and k % 4 == 0),
                    stop=last,
                    perf_mode=DR,
                )
            if last:
                t2_, half_ = q_ // 2, q_ % 2
                rmt_ = rmt_by_t2[t2_]
                nc.vector.tensor_copy(
                    rmt_[:, :KC, 128 * half_ : 128 * (half_ + 1)], prt_[:]
                )
                if half_ == 1:
                    pending_mlp2.append((t2_, rmt_))

        def mlp2(t2, rmt):
            xt8, xr16, _msk = xt_by_t2.pop(t2)

            o1 = po1.tile([128, KC, 256], BF16, tag="o1", name=f"o1_{t2}")
            o18s4 = po1.tile([128, KC, 256], FP8, tag="o18s4", name=f"o18s4_{t2}")
            o18d16 = po1.tile([128, KC, 256], FP8, tag="o18d16", name=f"o18d16_{t2}")
            rr4 = po1.tile([128, KC, 256], FP8, tag="rr4", name=f"rr4_{t2}")
            for m in range(KC):
                pb = psMm2.tile([128, 256], F32, tag="pb", name=f"pa{t2}_{m}")
                ms = slice(128 * m, 128 * (m + 1))
                for k4 in range(K4):
                    ks = slice(2 * k4, 2 * k4 + 2)
                    nc.tensor.matmul(
                        pb[:], b1s_sb[:, ks, ms], xt8[:, ks, :],
                        start=(k4 == 0), stop=False, perf_mode=DR,
                    )
                for k4 in range(K4):
                    ks = slice(2 * k4, 2 * k4 + 2)
                    nc.tensor.matmul(
                        pb[:], b1p_sb[:, ks, ms], xr16[:, ks, :],
                        start=False, stop=False, perf_mode=DR,
                    )
                for k4 in range(K4):
                    ks = slice(2 * k4, 2 * k4 + 2)
                    nc.tensor.matmul(
                        pb[:], b1r_sb[:, ks, ms], xt8[:, ks, :],
                        start=False, stop=False, perf_mode=DR,
                    )
                # W3 split: the even tile's rm half (cols 0:128) was copied a
                # whole tile earlier; the odd half just landed, so do it last.
                # The 5th chunk-pair carries mask (x) u16 — the folded u-term.
                for k4 in range(K4 + 1):
                    ks = slice(2 * k4, 2 * k4 + 2)
                    nc.tensor.matmul(
                        pb[:, 0:128], w3s_sb[:, ks, ms], rmt[:, ks, 0:128],
                        start=False, stop=False, perf_mode=DR,
                    )
                for k4 in range(K4 + 1):
                    ks = slice(2 * k4, 2 * k4 + 2)
                    nc.tensor.matmul(
                        pb[:, 128:256], w3s_sb[:, ks, ms], rmt[:, ks, 128:256],
                        start=False, stop=(k4 == K4), perf_mode=DR,
                    )
                nc.scalar.activation(o1[:, m, :], pb[:], AF.Relu,
                                     bias=b2a_sb[:, m : m + 1])
                # fp8 forms for the o2 residual matmuls: o18s4 = fp8(4*o1s16)
                # (= 64*o1_q), o18d16 = fp8(o1s16/16) (= o1_q), rr4 =
                # 4*o1s16 - o18s4 (= 64*(o1 - o1_q)). Emitted per m-PAIR as
                # double-width ops to halve the fixed per-op overhead.
                if m % 2 == 1:
                    mp = slice(m - 1, m + 1)
                    nc.scalar.activation(o18s4[:, mp, :], o1[:, mp, :],
                                         AF.Identity, scale=4.0)
                    nc.scalar.activation(o18d16[:, mp, :], o1[:, mp, :],
                                         AF.Identity, scale=1.0 / 16)
                    nc.vector.scalar_tensor_tensor(
                        out=rr4[:, mp, :], in0=o1[:, mp, :], scalar=4.0,
                        in1=o18s4[:, mp, :], op0=mybir.AluOpType.mult,
                        op1=mybir.AluOpType.subtract,
                    )

            o2 = po1.tile([128, KC, 256], BF16, tag="o2", name=f"o2_{t2}",
                          bufs=3)
            for m in range(KC):
                pb = psMm2.tile([128, 256], F32, tag="pb", name=f"pb{t2}_{m}")
                ms = slice(128 * m, 128 * (m + 1))
                for k4 in range(K4):
                    ks = slice(2 * k4, 2 * k4 + 2)
                    nc.tensor.matmul(
                        pb[:], w2b8_sb[:, ks, ms], o18s4[:, ks, :],
                        start=(k4 == 0), stop=False, perf_mode=DR,
                    )
                for k4 in range(K4):
                    ks = slice(2 * k4, 2 * k4 + 2)
                    nc.tensor.matmul(
                        pb[:], w2b8_sb[:, ks, ms], rr4[:, ks, :],
                        start=False, stop=False, perf_mode=DR,
                    )
                for k4 in range(K4):
                    ks = slice(2 * k4, 2 * k4 + 2)
                    nc.tensor.matmul(
                        pb[:], w2br_sb[:, ks, ms], o18d16[:, ks, :],
                        start=False, stop=(k4 == K4 - 1), perf_mode=DR,
                    )
                nc.scalar.activation(o2[:, m, :], pb[:], AF.Identity,
                                     bias=b2b_sb[:, m : m + 1], scale=1.0 / 64)
                if t2 == NT // 2 - 1:
                    nc.sync.dma_start(
                        out_myT[:, m : m + 1, 256 * t2 : 256 * (t2 + 1)],
                        o2[:, m : m + 1, :],
                    )
            if t2 != NT // 2 - 1:
                nc.gpsimd.dma_start(out_myT[:, :, 256 * t2 : 256 * (t2 + 1)], o2[:])

        for pi, (q, js) in enumerate(pairs):
            t2, half = q // 2, q % 2
            first_in_q = js[0] == sum(pattern[:q])
            if half == 0 and first_in_q:
                rmt = prm.tile(
                    [128, KC + 2, 256], FP8, tag="rm", name=f"rm{t2}"
                )
                rmt_by_t2[t2] = rmt
                # chunk KC carries (mask x u16) for the folded u-term; KC+1
                # is a zero pad (both sides zero; fp8 garbage x 0 = NaN)
                nc.gpsimd.memset(rmt[:, KC : KC + 2, :], 0.0)
                prefetch_xt(t2)
                msk8 = xt_by_t2[t2][2]
                nc.gpsimd.tensor_copy(rmt[0:1, KC, :], msk8[:])
            if first_in_q:
                prt_by_q[q] = psPr.tile([128, KC, 128], F32, tag="pr", name=f"prt{q}")
            prt = prt_by_q[q]

            psb, pcol = divmod(pi, 4)
            if pcol == 0:
                hi = min(4, NPAIR - 4 * psb)
                sp_sb = ssp.tile([128, 4, 2, 128], FP8, tag="sp", name=f"sp{psb}")
                nc.sync.dma_start(
                    sp_sb[:, :hi, :, :], sp_d[:, 4 * psb : 4 * psb + hi, :, :]
                )
            spv = sp_sb[:, pcol, :, :]

            g2 = pg2.tile([128, 2, D], FP8, tag="g2", name=f"g2_{pi}")
            if len(js) == 1:
                # half pair: unused g half could hold fp8 NaN garbage; its sp
                # half is 0 but NaN*0 = NaN, so zero it
                nc.gpsimd.memset(g2[:, 1, :], 0.0)
            for gi, jj in enumerate(js):
                sb, col = divmod(jj, 4)
                if col == 0:
                    load_eat(sb + 2)   # 2-superblock lookahead (bufs=3)
                    eat = eat_tiles.pop(sb)
                xwg = pgat.tile([128, D], FP8, tag="xwg", name=f"xwg{jj}")
                nc.gpsimd.indirect_dma_start(
                    out=xwg[:],
                    out_offset=None,
                    in_=xw_dram[:],
                    in_offset=bass.IndirectOffsetOnAxis(
                        ap=srcidx_sb[:, jj : jj + 1], axis=0
                    ),
                )
                ph = psBig.tile([128, KC, 128], F32, tag="big", name=f"ph{jj}")
                for h in range(2):
                    for k4 in range(K4):
                        nc.tensor.matmul(
                            ph[:, 4 * h : 4 * (h + 1), :],
                            eat[:, 2 * k4 : 2 * k4 + 2,
                                128 * col : 128 * (col + 1)],
                            w1a_sb[:, KC + 2 * k4 : KC + 2 * k4 + 2,
                                   512 * h : 512 * (h + 1)],
                            start=(k4 == 0),
                            stop=(k4 == K4 - 1),
                            perf_mode=DR,
                        )
                gt = pg.tile([128, D], BF16, tag="gt", name=f"gt{jj}")
                nc.vector.tensor_tensor(out=gt[:], in0=ph[:], in1=xwg[:],
                                        op=mybir.AluOpType.add)
                nc.scalar.activation(g2[:, gi, :], gt[:], AF.Relu)

            is_first = js[0] == sum(pattern[:q])
            is_last = js[-1] == sum(pattern[: q + 1]) - 1
            target = pi - (DEFER0 if pi < DEFER0 + 5 else DEFER)
            while next_fin <= target:
                finalize(next_fin)
                next_fin += 1
            while pending_mlp2:
                mlp2(*pending_mlp2.pop(0))
            state[pi] = (g2, spv, prt, is_first, is_last, q)
        while next_fin < NPAIR:
            finalize(next_fin)
            next_fin += 1
            while pending_mlp2:
                mlp2(*pending_mlp2.pop(0))

    nc.compile()
    return nc


def _get_program(NG, GU, pattern):
    key = (NG, GU, tuple(pattern))
    if key not in _PROGRAM_CACHE:
        _PROGRAM_CACHE[key] = _build_program(NG, GU, tuple(pattern))
    return _PROGRAM_CACHE[key]


def _pack_core(deg):
    """Pack NPC nodes (weights deg) into NT bins, <=128 nodes each,
    minimizing sum(ceil(load/128)). Returns list of (nodes, load)."""
    order = np.argsort(-deg, kind="stable")
    nodes = [[] for _ in range(NT)]
    load = np.zeros(NT, np.int64)
    cnt = np.zeros(NT, np.int64)
    for n in order:
        # LPT with node cap
        cand = [b for b in range(NT) if cnt[b] < 128]
        b = min(cand, key=lambda b: (load[b], cnt[b]))
        nodes[b].append(n)
        load[b] += deg[n]
        cnt[b] += 1
    # refinement: reduce sum(ceil(load/128)) by moving small nodes out of
    # bins that spill just over a multiple of 128
    for _ in range(200):
        ceil = -(-load // 128)
        improved = False
        spill_key = np.where(
            (load > 0) & (load % 128 != 0), (load - 1) % 128 + 1, 10**9
        )
        for a in np.argsort(spill_key):
            if load[a] == 0 or (load[a] % 128) == 0:
                continue
            spill = load[a] - 128 * (ceil[a] - 1)
            # try to move small nodes (total <= spill) from a to other bins
            small = sorted((deg[n], n) for n in nodes[a] if deg[n] > 0)
            moved = []
            need = spill
            for d, n in small:
                if d > need:
                    break
                tgt = None
                for b in range(NT):
                    if b == a or cnt[b] >= 128:
                        continue
                    if -(-(load[b] + d) // 128) == ceil[b]:
                        tgt = b
                        break
                if tgt is None:
                    continue
                nodes[a].remove(n)
                nodes[tgt].append(n)
                load[a] -= d
                load[tgt] += d
                cnt[a] -= 1
                cnt[tgt] += 1
                moved.append(n)
                need -= d
                if need <= 0:
                    break
            if need <= 0 and moved:
                improved = True
                break
        if not improved:
            break
    return [(nodes[b], int(load[b])) for b in range(NT)]


def _chunk_major(mat):
    """[D, n] f32/other -> [128, KC, n] fp8 chunk-major: out[p, c, j] =
    mat[128c+p, j]."""
    Dd, n = mat.shape
    assert Dd == D
    return np.ascontiguousarray(
        mat.reshape(KC, 128, n).transpose(1, 0, 2)
    )


def _make_in_maps(x, edge_index, edge_attr, W1a, b1a, W1b, b1b, W2a, b2a, W2b, b2b):
    x = np.ascontiguousarray(np.asarray(x, np.float32))
    edge_attr = np.ascontiguousarray(np.asarray(edge_attr, np.float32))
    ei = np.asarray(edge_index)
    row, col = ei[0].astype(np.int64), ei[1].astype(np.int64)

    perm = np.argsort(col, kind="stable")
    col_s, row_s = col[perm], row[perm]
    core_bounds = np.searchsorted(col_s, NPC * np.arange(C + 1))
    counts = np.bincount(col, minlength=N)

    # ---- pack nodes per core; derive the shared group pattern ----
    packs, uniqs = [], []
    for c in range(C):
        deg = counts[NPC * c : NPC * (c + 1)]
        bins = _pack_core(deg)
        bins.sort(key=lambda bl: -bl[1])
        packs.append(bins)
        s0, e0 = core_bounds[c], core_bounds[c + 1]
        uniqs.append(np.unique(row_s[s0:e0]))
    pattern = tuple(
        int(max(-(-packs[c][q][1] // 128) for c in range(C))) for q in range(NT)
    )
    pattern = tuple(max(p, 1) for p in pattern)
    NG = sum(pattern)
    GU = max(1, -(-max(len(u) for u in uniqs) // 128))
    UP, S = GU * 128, NG * 128
    pairs = _pair_schedule(pattern)
    NPAIR = len(pairs)
    # map group jj -> (pair index, half within pair)
    pair_idx_of_jj = np.zeros(NG, np.int64)
    half_of_jj = np.zeros(NG, np.int64)
    for pi, (q, js) in enumerate(pairs):
        for gi, jj in enumerate(js):
            pair_idx_of_jj[jj] = pi
            half_of_jj[jj] = gi

    # ---- fold weights (float64 for accuracy) ----
    B1 = np.asarray(W2a, np.float64)[:D].astype(np.float32)
    B2 = np.asarray(W2a, np.float64)[D:]
    W3 = (np.asarray(W1b, np.float64) @ B2).astype(np.float32)
    u_vec = (np.asarray(b1b, np.float64) @ B2).astype(np.float32)

    # fp8 residual split of B1 (x@B1 = x8@B1_8s16/16 + xr16@B1_8/16 + x8@B1_r16/16)
    B1_8 = B1.astype(E4M3)
    B1_8s16 = (16.0 * B1_8.astype(np.float32)).astype(E4M3)
    B1_r16 = (16.0 * (B1 - B1_8.astype(np.float32))).astype(E4M3)
    W3_s16 = (16.0 * W3).astype(E4M3)

    x8 = x.astype(E4M3)
    xr16 = (16.0 * (x - x8.astype(np.float32))).astype(E4M3)
    ea8 = edge_attr.astype(E4M3)

    # W3 chunk-major with 2 extra chunks: [p, KC, n] = u16[n] at p==0 (the
    # folded u-term, contracted against the mask row in rm chunk KC)
    W2bf = np.asarray(W2b, np.float32)
    W2b8 = W2bf.astype(E4M3)
    w2b8_cm = _chunk_major(W2b8)
    w2br_cm = _chunk_major((64.0 * (W2bf - W2b8.astype(np.float32))).astype(E4M3))

    w3s_ext = np.zeros((128, KC + 2, D), E4M3)
    w3s_ext[:, :KC, :] = _chunk_major(W3_s16)
    w3s_ext[0, KC, :] = (16.0 * u_vec).astype(E4M3)

    # w1a is [2D, D]: build [128, 2*KC, D]
    w1a8 = np.asarray(W1a, np.float32).astype(E4M3)
    w1a3 = np.ascontiguousarray(
        w1a8.reshape(2 * KC, 128, D).transpose(1, 0, 2)
    )
    b1a2 = np.zeros((1, 2, D), E4M3)
    b1a2[0, 0, :] = np.asarray(b1a, np.float32).astype(E4M3)
    ones2 = np.ones((1, 2, 128), E4M3)

    in_maps = []
    orders = []
    for c in range(C):
        s0 = core_bounds[c]
        lo = NPC * c
        bins = packs[c]
        uniq = uniqs[c]
        deg = counts[lo : lo + NPC]
        starts = np.zeros(NPC + 1, np.int64)
        np.cumsum(deg, out=starts[1:])

        src_l = np.zeros((128, NG), np.int32)     # local xw row per slot
        sp = np.zeros((128, NPAIR, 2, 128), E4M3)  # paired selection matrices
        ea_sel = np.full(S, -1, np.int64)         # edge_attr row per slot
        order = np.full(NP, -1, np.int64)         # packed node order

        goff = 0
        for q in range(NT):
            bnodes, load = bins[q]
            Gq = pattern[q]
            pos = 0
            for p, n in enumerate(bnodes):
                order[128 * q + p] = n
                d = int(deg[n])
                if d == 0:
                    continue
                ids = np.arange(starts[n], starts[n + 1], dtype=np.int64)
                sl = goff * 128 + pos + np.arange(d)
                gidx_, ridx_ = sl // 128, sl % 128
                src_l[ridx_, gidx_] = np.searchsorted(uniq, row_s[s0 + ids])
                sp[ridx_, pair_idx_of_jj[gidx_], half_of_jj[gidx_], p] = E4M3(
                    1.0 / d
                )
                ea_sel[sl] = perm[s0 + ids]
                pos += d
            assert pos <= 128 * Gq, (c, q, pos, Gq)
            goff += Gq
        assert goff == NG

        # chunk-major transposed, slot-ordered edge features (fp8)
        eaT = np.zeros((D, S), E4M3)
        nz = ea_sel >= 0
        eaT[:, nz] = ea8[ea_sel[nz]].T
        eat3 = _chunk_major(eaT)
        # chunk-major transposed unique-source features (fp8)
        xtuT = np.zeros((D, UP), E4M3)
        xtuT[:, : len(uniq)] = x8[uniq].T
        xtu3 = _chunk_major(xtuT)

        ordc = np.maximum(order, 0)
        valid = order >= 0
        cnt_c = np.where(valid, deg[ordc], 0)
        mask_c = ((cnt_c > 0) & valid).astype(E4M3)
        x8_c = np.where(valid[:, None], x8[lo + ordc], E4M3(0.0))
        xr16_c = np.where(valid[:, None], xr16[lo + ordc], E4M3(0.0))

        in_maps.append(
            {
                "xtu_d": xtu3,
                "eat_d": eat3,
                "sp_d": sp,
                "srcidx": src_l,
                "x8t_d": np.ascontiguousarray(_chunk_major(np.ascontiguousarray(x8_c.T)).reshape(128, KC, NT // 2, 256).transpose(0, 2, 1, 3)),
                "xr16t_d": np.ascontiguousarray(_chunk_major(np.ascontiguousarray(xr16_c.T)).reshape(128, KC, NT // 2, 256).transpose(0, 2, 1, 3)),
                "maskv8": mask_c.reshape(1, NP),
                "ones2_d": ones2,
                "w1a_d": w1a3,
                "b1s_d": _chunk_major(B1_8s16.astype(E4M3)),
                "b1p_d": _chunk_major(B1_8),
                "b1r_d": _chunk_major(B1_r16),
                "w3s_d": w3s_ext,
                "w2b8_d": w2b8_cm,
                "w2br_d": w2br_cm,
                "b1a2_d": b1a2,
                "b2a_d": 16.0 * np.asarray(b2a, np.float32).reshape(KC, 128).T,
                "b2b_d": np.asarray(b2b, np.float32).reshape(KC, 128).T.copy(),
            }
        )
        orders.append(order)
    return (NG, GU, pattern), in_maps, orders


def kernel(x, edge_index, edge_attr, W1a, b1a, W1b, b1b, W2a, b2a, W2b, b2b):
    args, in_maps, orders = _make_in_maps(
        x, edge_index, edge_attr, W1a, b1a, W1b, b1b, W2a, b2a, W2b, b2b
    )
    nc = _get_program(*args)
    res = run_bass_kernel_spmd(nc, in_maps, core_ids=list(range(C)))
    out = np.empty((N, D), np.float32)
    for c in range(C):
        o = np.asarray(res.results[c]["out_myT"]).astype(np.float32)
        o = o.transpose(2, 1, 0).reshape(NP, D)  # [n, 128c+p] from [p, c, n]
        order = orders[c]
        valid = order >= 0
        out[NPC * c + order[valid]] = o[valid]
    return np.ascontiguousarray(out)
